# revision 2
# baseline (speedup 1.0000x reference)
"""Trainium2 Bass kernel: hard-negative miner (masked top-5 indices over 50257 classes).

Data-parallel over 8 NeuronCores (1024 rows each). Per 128-row tile:
  1. Stream the padded row (50432 cols) through SBUF in 4 column blocks;
     windowed tensor_reduce(max) -> 197 subchunk maxes (L=256 per subchunk).
  2. InstMax + InstMaxIndex over the subchunk maxes -> top-8 subchunk ids;
     sort them ascending (odd-even network on tiny tiles).
  3. 8x indirect-DMA: gather those subchunks from DRAM (ascending column order
     so fp32 value ties resolve to the lower column index, matching stable top_k).
  4. InstMax + InstMaxIndex over the gathered 2048 values -> exact top-8
     positions; decode to global column indices (shift/and + 8-way one-hot over
     the sorted subchunk ids); drop the row's label via compare+cumsum+select;
     emit the first 5 as int32.

Tie correctness: top-6 elements of a row lie in the top-8 subchunks-by-max
(holds under 2-way fp32 ties, which are common here: jax threefry normals are
quantized), and first-match InstMaxIndex over an ascending-column gather
reproduces stable (lowest-index-first) ordering.
"""

import sys

sys.path.insert(0, "/opt/trn_rl_repo")

import numpy as np

import concourse.bass as bass
import concourse.mybir as mybir
from concourse import bacc, bass_utils
from concourse.tile import TileContext

# ---- problem constants (hardcoded; kernel.py must be self-contained) ----
B = 8192  # batch
N = 50257  # classes
TOP_K = 5
NCORES = 8
R = B // NCORES  # rows per core = 1024
P = 128  # SBUF partitions
T = R // P  # row tiles per core = 8
L = 256  # subchunk length
S = (N + L - 1) // L  # subchunks per row = 197
W = S * L  # padded row width = 50432
NEG = -1.0e30

F32 = mybir.dt.float32
I32 = mybir.dt.int32
U32 = mybir.dt.uint32
AX = mybir.AxisListType.X
OP = mybir.AluOpType

_BLK_S = [50, 49, 49, 49]  # subchunks per streaming block
assert sum(_BLK_S) == S


def _tile_body(nc, tc, pp, pb, pe, t, x, lab8, base_t, out):
    """Emit all instructions for one 128-row tile."""
    # ---- pass 1: stream blocks, windowed max ----
    m_t = pe.tile([P, S], F32, tag="m")
    c0 = 0
    for ws in _BLK_S:
        blk = pb.tile([P, ws * L], F32, tag="blk")
        nc.sync.dma_start(
            out=blk[:, :],
            in_=x[t * P : (t + 1) * P, c0 * L : (c0 + ws) * L],
        )
        nc.vector.tensor_reduce(
            out=m_t[:, c0 : c0 + ws],
            in_=blk[:].rearrange("p (s l) -> p s l", l=L),
            axis=AX,
            op=OP.max,
        )
        c0 += ws

    # ---- top-8 subchunks ----
    c8 = pe.tile([P, 8], F32, tag="c8")
    pidx = pe.tile([P, 8], U32, tag="pidx")
    nc.vector.max(out=c8[:, :], in_=m_t[:, :])
    nc.vector.max_index(out=pidx[:, :], in_max=c8[:, :], in_values=m_t[:, :])

    # sort subchunk ids ascending: odd-even transposition, ping-pong buffers
    pa = pe.tile([P, 8], F32, tag="pa")
    pb2 = pe.tile([P, 8], F32, tag="pb2")
    nc.vector.tensor_copy(out=pa[:, :], in_=pidx[:, :])
    cur, nxt = pa, pb2
    for r in range(8):
        if r % 2 == 0:
            nc.vector.tensor_tensor(
                out=nxt[:, 0::2], in0=cur[:, 0::2], in1=cur[:, 1::2], op=OP.min
            )
            nc.vector.tensor_tensor(
                out=nxt[:, 1::2], in0=cur[:, 0::2], in1=cur[:, 1::2], op=OP.max
            )
        else:
            nc.vector.tensor_tensor(
                out=nxt[:, 1:7:2], in0=cur[:, 1:7:2], in1=cur[:, 2:8:2], op=OP.min
            )
            nc.vector.tensor_tensor(
                out=nxt[:, 2:8:2], in0=cur[:, 1:7:2], in1=cur[:, 2:8:2], op=OP.max
            )
            nc.vector.tensor_copy(out=nxt[:, 0::7], in_=cur[:, 0::7])
        cur, nxt = nxt, cur
    psort = cur  # fp32 subchunk ids, ascending

    # gather offsets: (t*128+p)*S + subchunk  (exact in fp32, < 2^24)
    offs_f = pe.tile([P, 8], F32, tag="offs_f")
    offs_i = pe.tile([P, 8], I32, tag="offs_i")
    nc.vector.tensor_tensor(
        out=offs_f[:, :],
        in0=psort[:, :],
        in1=base_t[:, t : t + 1].to_broadcast([P, 8]),
        op=OP.add,
    )
    nc.vector.tensor_copy(out=offs_i[:, :], in_=offs_f[:, :])

    # ---- gather the 8 winning subchunks per row (per-partition offsets) ----
    g_t = pe.tile([P, 8 * L], F32, tag="g")
    xflat = x.ap().rearrange("r (s l) -> (r s) l", l=L)
    for j in range(8):
        nc.gpsimd.indirect_dma_start(
            out=g_t[:, j * L : (j + 1) * L],
            out_offset=None,
            in_=xflat,
            in_offset=bass.IndirectOffsetOnAxis(ap=offs_i[:, j : j + 1], axis=0),
        )

    # ---- exact top-8 over gathered ----
    g8 = pe.tile([P, 8], F32, tag="g8")
    q = pe.tile([P, 8], U32, tag="q")
    nc.vector.max(out=g8[:, :], in_=g_t[:, :])
    nc.vector.max_index(out=q[:, :], in_max=g8[:, :], in_values=g_t[:, :])

    # decode gathered positions -> global column indices
    sid_u = pe.tile([P, 8], U32, tag="sid_u")
    win_u = pe.tile([P, 8], U32, tag="win_u")
    nc.vector.tensor_scalar(
        out=sid_u[:, :], in0=q[:, :], scalar1=int(L).bit_length() - 1,
        scalar2=None, op0=OP.logical_shift_right,
    )
    nc.vector.tensor_scalar(
        out=win_u[:, :], in0=q[:, :], scalar1=L - 1, scalar2=None,
        op0=OP.bitwise_and,
    )
    sid_f = pe.tile([P, 8], F32, tag="sid_f")
    win_f = pe.tile([P, 8], F32, tag="win_f")
    nc.vector.tensor_copy(out=sid_f[:, :], in_=sid_u[:, :])
    nc.vector.tensor_copy(out=win_f[:, :], in_=win_u[:, :])

    # chunk base = psort[sid] via 8-way one-hot
    acc = pe.tile([P, 8], F32, tag="acc")
    eqt = pe.tile([P, 8], F32, tag="eqt")
    trm = pe.tile([P, 8], F32, tag="trm")
    for c in range(8):
        nc.vector.tensor_scalar(
            out=eqt[:, :], in0=sid_f[:, :], scalar1=float(c), scalar2=None,
            op0=OP.is_equal,
        )
        pc = psort[:, c : c + 1].to_broadcast([P, 8])
        dst = acc if c == 0 else trm
        nc.vector.tensor_tensor(out=dst[:, :], in0=eqt[:, :], in1=pc, op=OP.mult)
        if c > 0:
            nc.vector.tensor_tensor(
                out=acc[:, :], in0=acc[:, :], in1=trm[:, :], op=OP.add
            )
    gidx = pe.tile([P, 8], F32, tag="gidx")
    nc.vector.tensor_scalar(
        out=acc[:, :], in0=acc[:, :], scalar1=float(L), scalar2=None, op0=OP.mult
    )
    nc.vector.tensor_tensor(
        out=gidx[:, :], in0=acc[:, :], in1=win_f[:, :], op=OP.add
    )

    # drop label, select first 5
    valid = pe.tile([P, 8], F32, tag="valid")
    nc.vector.tensor_tensor(
        out=valid[:, :],
        in0=gidx[:, :],
        in1=lab8[:, t : t + 1].to_broadcast([P, 8]),
        op=OP.not_equal,
    )
    c1 = pe.tile([P, 8], F32, tag="c1")
    c2 = pe.tile([P, 8], F32, tag="c2")
    c4 = pe.tile([P, 8], F32, tag="c4")
    nc.vector.tensor_copy(out=c1[:, :], in_=valid[:, :])
    nc.vector.tensor_tensor(
        out=c1[:, 1:8], in0=valid[:, 1:8], in1=valid[:, 0:7], op=OP.add
    )
    nc.vector.tensor_copy(out=c2[:, :], in_=c1[:, :])
    nc.vector.tensor_tensor(
        out=c2[:, 2:8], in0=c1[:, 2:8], in1=c1[:, 0:6], op=OP.add
    )
    nc.vector.tensor_copy(out=c4[:, :], in_=c2[:, :])
    nc.vector.tensor_tensor(
        out=c4[:, 4:8], in0=c2[:, 4:8], in1=c2[:, 0:4], op=OP.add
    )

    out5f = pe.tile([P, TOP_K], F32, tag="out5f")
    sel = pe.tile([P, 8], F32, tag="sel")
    for k in range(TOP_K):
        nc.vector.tensor_scalar(
            out=eqt[:, :], in0=c4[:, :], scalar1=float(k + 1), scalar2=None,
            op0=OP.is_equal,
        )
        nc.vector.tensor_tensor(
            out=sel[:, :], in0=eqt[:, :], in1=valid[:, :], op=OP.mult
        )
        nc.vector.tensor_tensor(
            out=sel[:, :], in0=sel[:, :], in1=gidx[:, :], op=OP.mult
        )
        nc.vector.tensor_reduce(
            out=out5f[:, k : k + 1], in_=sel[:, :], axis=AX, op=OP.add
        )
    out5i = pe.tile([P, TOP_K], I32, tag="out5i")
    nc.vector.tensor_copy(out=out5i[:, :], in_=out5f[:, :])
    nc.sync.dma_start(out=out[t * P : (t + 1) * P, :], in_=out5i[:, :])


def build_bass():
    nc = bacc.Bacc("TRN2", num_devices=NCORES)
    x = nc.dram_tensor("x", (R, W), F32, kind="ExternalInput")
    labf = nc.dram_tensor("labf", (R, 1), F32, kind="ExternalInput")
    basec = nc.dram_tensor("basec", (P, T), F32, kind="ExternalInput")
    out = nc.dram_tensor("out", (R, TOP_K), I32, kind="ExternalOutput")

    with TileContext(nc) as tc:
        with (
            tc.tile_pool(name="persist", bufs=1) as pp,
            tc.tile_pool(name="blk", bufs=3) as pb,
            tc.tile_pool(name="epi", bufs=2) as pe,
        ):
            lab8 = pp.tile([P, T], F32)
            base_t = pp.tile([P, T], F32)
            nc.sync.dma_start(
                out=lab8[:, :],
                in_=labf.ap().rearrange("(t p) one -> p (t one)", p=P),
            )
            nc.sync.dma_start(out=base_t[:, :], in_=basec[:, :])
            for t in range(T):
                _tile_body(nc, tc, pp, pb, pe, t, x, lab8, base_t, out)

    nc.compile()
    return nc


_NC_CACHE = None


def _get_nc():
    global _NC_CACHE
    if _NC_CACHE is None:
        _NC_CACHE = build_bass()
    return _NC_CACHE


def make_in_maps(teacher_logits: np.ndarray, labels: np.ndarray):
    """Host-side prep: pad, shard, build constants."""
    xpad = np.full((B, W), NEG, dtype=np.float32)
    xpad[:, :N] = teacher_logits
    labf = labels.astype(np.float32).reshape(B, 1)
    p = np.arange(P, dtype=np.float32).reshape(P, 1)
    t = np.arange(T, dtype=np.float32).reshape(1, T)
    basec = ((t * P + p) * S).astype(np.float32)  # (P, T)
    in_maps = []
    for c in range(NCORES):
        in_maps.append(
            {
                "x": xpad[c * R : (c + 1) * R],
                "labf": labf[c * R : (c + 1) * R],
                "basec": basec,
            }
        )
    return in_maps


def kernel(teacher_logits: np.ndarray, labels: np.ndarray) -> np.ndarray:
    nc = _get_nc()
    in_maps = make_in_maps(np.asarray(teacher_logits), np.asarray(labels))
    res = bass_utils.run_bass_kernel_spmd(nc, in_maps, core_ids=list(range(NCORES)))
    out = np.concatenate([r["out"] for r in res.results], axis=0)
    return out.astype(np.int32)


# revision 9
# speedup vs baseline: 158.0210x; 158.0210x over previous
"""Trainium2 Bass kernel: hard-negative miner (masked top-5 indices over 50257 classes).

Data-parallel over 8 NeuronCores (1024 rows each). Per 128-row tile:
  1. Stream the padded row (50432 cols) through SBUF in column blocks;
     windowed tensor_reduce(max) -> 197 subchunk maxes (L=256 per subchunk).
  2. InstMax + InstMaxIndex over the subchunk maxes -> top-8 subchunk ids;
     sort them ascending (odd-even network on tiny tiles).
  3. 8x indirect-DMA: gather those subchunks from DRAM (ascending column order
     so fp32 value ties resolve to the lower column index, matching stable top_k).
Then one batched tail over all 8 tiles (keeps tiny dependent ops out of the
DMA-bound streaming pipeline):
  4. InstMax + InstMaxIndex over each tile's gathered 2048 values -> exact top-8
     positions; decode to global column indices (shift/and + 8-way one-hot over
     the sorted subchunk ids); drop the row's label via compare+cumsum+select;
     emit the first 5 as int32.

Tie correctness: top-6 elements of a row lie in the top-8 subchunks-by-max
(holds under 2-way fp32 ties, which are common here: jax threefry normals are
quantized), and first-match InstMaxIndex over an ascending-column gather
reproduces stable (lowest-index-first) ordering.
"""

import sys

sys.path.insert(0, "/opt/trn_rl_repo")

import numpy as np

import concourse.bass as bass
import concourse.mybir as mybir
from concourse import bacc, bass_utils
from concourse.tile import TileContext

# ---- problem constants (hardcoded; kernel.py must be self-contained) ----
B = 8192  # batch
N = 50257  # classes
TOP_K = 5
NCORES = 8
R = B // NCORES  # rows per core = 1024
P = 128  # SBUF partitions
T = R // P  # row tiles per core = 8
L = 256  # subchunk length
S = (N + L - 1) // L  # subchunks per row = 197
W = S * L  # padded row width = 50432
NEG = -1.0e30

F32 = mybir.dt.float32
I32 = mybir.dt.int32
U32 = mybir.dt.uint32
AX = mybir.AxisListType.X
OP = mybir.AluOpType

_BLK_S = [50, 49, 49, 49]  # subchunks per streaming block
assert sum(_BLK_S) == S


def _tile_stream(nc, pb, pe, t, x, base_t, psall, gall, alt_dma=True, dma_rot=2,
                 stop_after="all"):
    """Scan one 128-row tile, pick+sort its top-8 subchunks, gather them."""
    m_t = pe.tile([P, S], F32, tag="m")
    c0 = 0
    for bi, ws in enumerate(_BLK_S):
        blk = pb.tile([P, ws * L], F32, tag="blk")
        engs = [nc.sync, nc.scalar, nc.gpsimd][:dma_rot] if alt_dma else [nc.sync]
        eng = engs[bi % len(engs)]
        eng.dma_start(
            out=blk[:, :],
            in_=x[t * P : (t + 1) * P, c0 * L : (c0 + ws) * L],
        )
        nc.vector.tensor_reduce(
            out=m_t[:, c0 : c0 + ws],
            in_=blk[:].rearrange("p (s l) -> p s l", l=L),
            axis=AX,
            op=OP.max,
        )
        c0 += ws
    if stop_after == "scan":
        return

    c8 = pe.tile([P, 8], F32, tag="c8")
    pidx = pe.tile([P, 8], U32, tag="pidx")
    nc.vector.max(out=c8[:, :], in_=m_t[:, :])
    nc.vector.max_index(out=pidx[:, :], in_max=c8[:, :], in_values=m_t[:, :])

    # sort subchunk ids ascending: odd-even transposition, ping-pong buffers
    pa = pe.tile([P, 8], F32, tag="pa")
    pb2 = pe.tile([P, 8], F32, tag="pb2")
    nc.vector.tensor_copy(out=pa[:, :], in_=pidx[:, :])
    cur, nxt = pa, pb2
    for r in range(8):
        if r % 2 == 0:
            nc.vector.tensor_tensor(
                out=nxt[:, 0::2], in0=cur[:, 0::2], in1=cur[:, 1::2], op=OP.min
            )
            nc.vector.tensor_tensor(
                out=nxt[:, 1::2], in0=cur[:, 0::2], in1=cur[:, 1::2], op=OP.max
            )
        else:
            nc.vector.tensor_tensor(
                out=nxt[:, 1:7:2], in0=cur[:, 1:7:2], in1=cur[:, 2:8:2], op=OP.min
            )
            nc.vector.tensor_tensor(
                out=nxt[:, 2:8:2], in0=cur[:, 1:7:2], in1=cur[:, 2:8:2], op=OP.max
            )
            nc.vector.tensor_copy(out=nxt[:, 0::7], in_=cur[:, 0::7])
        cur, nxt = nxt, cur
    nc.vector.tensor_copy(out=psall[:, t * 8 : (t + 1) * 8], in_=cur[:, :])

    # gather offsets: (t*128+p)*S + subchunk  (exact in fp32, < 2^24)
    offs_f = pe.tile([P, 8], F32, tag="offs_f")
    offs_i = pe.tile([P, 8], I32, tag="offs_i")
    nc.vector.tensor_tensor(
        out=offs_f[:, :],
        in0=cur[:, :],
        in1=base_t[:, t : t + 1].to_broadcast([P, 8]),
        op=OP.add,
    )
    nc.vector.tensor_copy(out=offs_i[:, :], in_=offs_f[:, :])

    if stop_after == "cand":
        return
    xflat = x.ap().rearrange("r (s l) -> (r s) l", l=L)
    for j in range(8):
        nc.gpsimd.indirect_dma_start(
            out=gall[:, t * 8 * L + j * L : t * 8 * L + (j + 1) * L],
            out_offset=None,
            in_=xflat,
            in_offset=bass.IndirectOffsetOnAxis(ap=offs_i[:, j : j + 1], axis=0),
        )


def _tile_scans(nc, t, g8all, qall, gall):
    nc.vector.max(
        out=g8all[:, t * 8 : (t + 1) * 8],
        in_=gall[:, t * 8 * L : (t + 1) * 8 * L],
    )
    nc.vector.max_index(
        out=qall[:, t * 8 : (t + 1) * 8],
        in_max=g8all[:, t * 8 : (t + 1) * 8],
        in_values=gall[:, t * 8 * L : (t + 1) * 8 * L],
    )


def _final_tail(nc, pe, lab8, psall, gall, out, g8all, qall, t0=0, nt=T,
                do_scans=True):
    """Exact top-8 per tile group, decode, label-drop, store — for the
    contiguous tile range [t0, t0+nt)."""
    if do_scans:
        for i in range(nt):
            _tile_scans(nc, t0 + i, g8all, qall, gall)
    q = qall[:, t0 * 8 : (t0 + nt) * 8]

    def r3(ap):
        return ap[:].rearrange("p (t j) -> p t j", j=8)

    # decode gathered positions -> global column indices
    sid_u = pe.tile([P, nt * 8], U32, tag="sid_u")
    win_u = pe.tile([P, nt * 8], U32, tag="win_u")
    nc.vector.tensor_scalar(
        out=sid_u[:, :], in0=q, scalar1=int(L).bit_length() - 1,
        scalar2=None, op0=OP.logical_shift_right,
    )
    nc.vector.tensor_scalar(
        out=win_u[:, :], in0=q, scalar1=L - 1, scalar2=None,
        op0=OP.bitwise_and,
    )
    sid_f = pe.tile([P, nt * 8], F32, tag="sid_f")
    win_f = pe.tile([P, nt * 8], F32, tag="win_f")
    nc.vector.tensor_copy(out=sid_f[:, :], in_=sid_u[:, :])
    nc.vector.tensor_copy(out=win_f[:, :], in_=win_u[:, :])

    # chunk base = psort[sid] via 8-way one-hot (batched across the range)
    ps3 = psall[:].rearrange("p (t j) -> p t j", j=8)[:, t0 : t0 + nt, :]
    acc = pe.tile([P, nt * 8], F32, tag="acc")
    eqt = pe.tile([P, nt * 8], F32, tag="eqt")
    trm = pe.tile([P, nt * 8], F32, tag="trm")
    for c in range(8):
        nc.vector.tensor_scalar(
            out=eqt[:, :], in0=sid_f[:, :], scalar1=float(c), scalar2=None,
            op0=OP.is_equal,
        )
        pc = ps3[:, :, c : c + 1].to_broadcast([P, nt, 8])
        dst = acc if c == 0 else trm
        nc.vector.tensor_tensor(out=r3(dst), in0=r3(eqt), in1=pc, op=OP.mult)
        if c > 0:
            nc.vector.tensor_tensor(
                out=acc[:, :], in0=acc[:, :], in1=trm[:, :], op=OP.add
            )
    gidx = pe.tile([P, nt * 8], F32, tag="gidx")
    nc.vector.tensor_scalar(
        out=acc[:, :], in0=acc[:, :], scalar1=float(L), scalar2=None, op0=OP.mult
    )
    nc.vector.tensor_tensor(
        out=gidx[:, :], in0=acc[:, :], in1=win_f[:, :], op=OP.add
    )

    # drop label, select first 5 per tile group
    valid = pe.tile([P, nt * 8], F32, tag="valid")
    lab3 = lab8[:].rearrange("p (t one) -> p t one", one=1)
    nc.vector.tensor_tensor(
        out=r3(valid),
        in0=r3(gidx),
        in1=lab3[:, t0 : t0 + nt, 0:1].to_broadcast([P, nt, 8]),
        op=OP.not_equal,
    )
    c1 = pe.tile([P, nt * 8], F32, tag="c1")
    c2 = pe.tile([P, nt * 8], F32, tag="c2")
    c4 = pe.tile([P, nt * 8], F32, tag="c4")
    v3, c13, c23, c43 = r3(valid), r3(c1), r3(c2), r3(c4)
    nc.vector.tensor_copy(out=c1[:, :], in_=valid[:, :])
    nc.vector.tensor_tensor(
        out=c13[:, :, 1:8], in0=v3[:, :, 1:8], in1=v3[:, :, 0:7], op=OP.add
    )
    nc.vector.tensor_copy(out=c2[:, :], in_=c1[:, :])
    nc.vector.tensor_tensor(
        out=c23[:, :, 2:8], in0=c13[:, :, 2:8], in1=c13[:, :, 0:6], op=OP.add
    )
    nc.vector.tensor_copy(out=c4[:, :], in_=c2[:, :])
    nc.vector.tensor_tensor(
        out=c43[:, :, 4:8], in0=c23[:, :, 4:8], in1=c23[:, :, 0:4], op=OP.add
    )

    out5f = pe.tile([P, nt * TOP_K], F32, tag="out5f")
    out5f3 = out5f[:].rearrange("p (t k) -> p t k", k=TOP_K)
    sel = pe.tile([P, nt * 8], F32, tag="sel")
    for k in range(TOP_K):
        nc.vector.tensor_scalar(
            out=eqt[:, :], in0=c4[:, :], scalar1=float(k + 1), scalar2=None,
            op0=OP.is_equal,
        )
        nc.vector.tensor_tensor(
            out=sel[:, :], in0=eqt[:, :], in1=valid[:, :], op=OP.mult
        )
        nc.vector.tensor_tensor(
            out=sel[:, :], in0=sel[:, :], in1=gidx[:, :], op=OP.mult
        )
        nc.vector.tensor_reduce(
            out=out5f3[:, :, k : k + 1], in_=r3(sel), axis=AX, op=OP.add
        )
    out5i = pe.tile([P, nt * TOP_K], I32, tag="out5i")
    nc.vector.tensor_copy(out=out5i[:, :], in_=out5f[:, :])
    nc.sync.dma_start(
        out=out.ap().rearrange("(t p) k -> p t k", p=P)[:, t0 : t0 + nt, :],
        in_=out5i[:].rearrange("p (t k) -> p t k", k=TOP_K),
    )


def build_bass(rep: int = 1, lag: int = 0, alt_dma: bool = True,
               scan_lag: int = 0, dma_rot: int = 2, stop_after: str = "all"):
    """scan_lag>0: emit per-tile gathered-max scans `scan_lag` tiles behind the
    stream; decode stays batched at the end. dma_rot: 2=sync/scalar alternation,
    3=sync/scalar/gpsimd rotation for block loads."""
    nc = bacc.Bacc("TRN2", num_devices=NCORES)
    x = nc.dram_tensor("x", (R, W), F32, kind="ExternalInput")
    labf = nc.dram_tensor("labf", (R, 1), F32, kind="ExternalInput")
    basec = nc.dram_tensor("basec", (P, T), F32, kind="ExternalInput")
    out = nc.dram_tensor("out", (R, TOP_K), I32, kind="ExternalOutput")

    with TileContext(nc) as tc:
        with (
            tc.tile_pool(name="persist", bufs=1) as pp,
            tc.tile_pool(name="blk", bufs=2) as pb,
            tc.tile_pool(name="epi", bufs=2) as pe,
        ):
            lab8 = pp.tile([P, T], F32)
            base_t = pp.tile([P, T], F32)
            nc.sync.dma_start(
                out=lab8[:, :],
                in_=labf.ap().rearrange("(t p) one -> p (t one)", p=P),
            )
            nc.sync.dma_start(out=base_t[:, :], in_=basec[:, :])
            for _ in range(rep):
                psall = pp.tile([P, T * 8], F32, tag="psall")
                gall = pp.tile([P, T * 8 * L], F32, tag="gall")
                g8all = pp.tile([P, T * 8], F32, tag="g8all")
                qall = pp.tile([P, T * 8], U32, tag="qall")
                for t in range(T):
                    _tile_stream(nc, pb, pe, t, x, base_t, psall, gall,
                                 alt_dma=alt_dma, dma_rot=dma_rot,
                                 stop_after=stop_after)
                    if stop_after == "all" and scan_lag and t >= scan_lag:
                        _tile_scans(nc, t - scan_lag, g8all, qall, gall)
                if stop_after == "all" and scan_lag:
                    for t in range(T - scan_lag, T):
                        _tile_scans(nc, t, g8all, qall, gall)
                if stop_after == "all":
                    _final_tail(nc, pe, lab8, psall, gall, out, g8all, qall,
                                t0=0, nt=T, do_scans=not scan_lag)

    nc.compile()
    return nc


_NC_CACHE = None


def _get_nc():
    global _NC_CACHE
    if _NC_CACHE is None:
        _NC_CACHE = build_bass()
    return _NC_CACHE


def make_in_maps(teacher_logits: np.ndarray, labels: np.ndarray):
    """Host-side prep: pad, shard, build constants."""
    xpad = np.full((B, W), NEG, dtype=np.float32)
    xpad[:, :N] = teacher_logits
    labf = labels.astype(np.float32).reshape(B, 1)
    p = np.arange(P, dtype=np.float32).reshape(P, 1)
    t = np.arange(T, dtype=np.float32).reshape(1, T)
    basec = ((t * P + p) * S).astype(np.float32)  # (P, T)
    in_maps = []
    for c in range(NCORES):
        in_maps.append(
            {
                "x": xpad[c * R : (c + 1) * R],
                "labf": labf[c * R : (c + 1) * R],
                "basec": basec,
            }
        )
    return in_maps


def kernel(teacher_logits: np.ndarray, labels: np.ndarray) -> np.ndarray:
    nc = _get_nc()
    in_maps = make_in_maps(np.asarray(teacher_logits), np.asarray(labels))
    res = bass_utils.run_bass_kernel_spmd(nc, in_maps, core_ids=list(range(NCORES)))
    out = np.concatenate([r["out"] for r in res.results], axis=0)
    return out.astype(np.int32)
